# revision 20
# baseline (speedup 1.0000x reference)
"""CapsuleLayer dynamic-routing kernel for Trainium2 (8 NeuronCores).

Problem (hardcoded):
  inputs: [B=16, I=1152, Din=16] f32
  W:      [1, N=32, I=1152, D=64, Din=16] f32
  x_hat = einsum('nidk,bik->bnid', W[0], inputs)        # [B,N,I,D]
  3 routing iterations of per-(b,n,d) softmax over I (size-1-dim squash
  quirk makes everything elementwise in d), output [B,N,D,1] f32.

Key algebra:
  * iter0: softmax(0) uniform -> s0 = mean_i(x_hat); V accumulates squash
    outputs so logits are x_hat * V (never materialized).
  * iter1 via MOMENTS: z = V0*x_hat is small (|z| <= ~2.8 on this data;
    99.9% < 0.77) because V0 = squash(mean_i x_hat / I) ~ O(0.01).  Taylor:
      denom = sum_i e^z   ~= I + V0*S1 + (V0^2/2)*S2
      numer = sum_i x*e^z ~= S1 + V0*S2
    with S1 = sum_i x_hat (the existing mean matmul) and S2 = sum_i x_hat^2
    (one elementwise pass, replacing iter1's STT pass 1:1).  Verified on the
    reference data: final rel err 4.4e-3 vs 4.1e-3 for exact iter1.
  * iter2 exact: E = exp(V1*x_hat) on ACT (accum_out -> denom), DVE
    scalar_tensor_tensor P=(E*1)*x_hat with accum_out -> numer.
  * squash(s) = s*|s|/(1+s^2), with |s| ~= sqrt(s^2+1e-9) via |s+1e-20|.

Mapping (per core; N sharded 8 ways, 4 capsules = 2 "pairs" of (2n x 64d)):
  * x_hat gen: stationary = W slab [(ig,k)=128, (n2,d)=128]; moving =
    block-diagonal input [(ig,k)=128, (b,ig')=128], one matmul per i-block.
    The i-sum S1 accumulates in PSUM from a second matmul per block against
    a DENSE input operand [(ig,k)=128, b=16].
  * PSUM evacuated in [128, 12x128] chunks f32->bf16.  ALL pair-0 copies go
    on ACT (they hide under pair-0's input DMA, and DVE stays free to start
    the S2 pass the moment X half-tiles land); pair-1's copies go 9 ACT / 3
    DVE while DVE runs pair-0's S2/STT stream.
  * S2 slices split DVE (STT x^2 + accum) / ACT (Square activation + accum)
    to balance; pair-1's S2 interleaves with pair-0's iter2 stream.
  * small [128,16] chain ops (poly eval, squash pieces) go to Pool where
    legal (tensor_tensor only); reciprocal stays on DVE; Abs on ACT.
  * outputs DMA via Pool SWDGE; final segment splits the reduce chain in
    batch halves so the first half's squash+DMA hides under the second.
"""

import numpy as np

# ---------------- problem constants (hardcoded per contract) ----------------
B, I, DIN = 16, 1152, 16
N, D = 32, 64
NCORES = 8
NL = N // NCORES        # 4 capsules per core
NPAIR = NL // 2         # 2 capsule-pairs per core (2 n's x 64 d = 128 parts)
IG = 8                  # i's folded into the contraction dim
NBLK = I // IG          # 144 i-blocks
CHUNK = 24              # i-blocks per DMA super-tile
NCHUNK = NBLK // CHUNK  # 6
GRP = 12                # i-blocks per PSUM evacuation tile (3 banks)

_compiled = {}


def _build_program(stage="full", reps=1, act_copy_mod=1, p1_act_evac=9,
                   q1=8, q0=0, epool_bufs=8, ppool_bufs=4, spool_bufs=12,
                   wsup_bufs=4, out_q=2, split_tail=True, use_moments=True,
                   interleave=2, inpd_act=2, wsup1_pool=3):
    import concourse.bacc as bacc
    import concourse.mybir as mybir
    import concourse.tile as tile

    f32 = mybir.dt.float32
    bf16 = mybir.dt.bfloat16
    Alu = mybir.AluOpType
    Act = mybir.ActivationFunctionType

    nc = bacc.Bacc("TRN2", target_bir_lowering=False, debug=False)

    wslab_d = nc.declare_dram_parameter(
        "wslab", [NPAIR, NCHUNK, 128, CHUNK, 128], bf16, isOutput=False)
    inpblk_d = nc.declare_dram_parameter(
        "inpblk", [NCHUNK, 128, CHUNK, 128], bf16, isOutput=False)
    dens_d = nc.declare_dram_parameter(
        "dens", [128, NBLK, B], bf16, isOutput=False)
    out_d = nc.declare_dram_parameter(
        "out", [NPAIR, 128, B], f32, isOutput=True)

    with tile.TileContext(nc) as tc:
        with (
            tc.tile_pool(name="persist", bufs=1) as xpool,
            tc.tile_pool(name="wsup", bufs=wsup_bufs) as wpool,
            tc.tile_pool(name="escr", bufs=epool_bufs) as epool,
            tc.tile_pool(name="pscr", bufs=ppool_bufs) as ppool,
            tc.tile_pool(name="small", bufs=spool_bufs) as spool,
            tc.tile_pool(name="psum", bufs=2, space="PSUM") as psum,
            tc.tile_pool(name="psmean", bufs=1, space="PSUM") as psmean,
        ):
            # X free layout: (blk, col) with col = b*IG + ig
            X = [xpool.tile([128, NBLK, 128], bf16, tag=f"X{p}",
                            name=f"X{p}") for p in range(NPAIR)]
            inpD = xpool.tile([128, NBLK, 128], bf16, tag="inpD", name="inpD")
            densT = xpool.tile([128, NBLK, B], bf16, tag="densT",
                               name="densT")

            epsb = xpool.tile([128, 1], f32, tag="epsb", name="epsb")
            nc.vector.memset(epsb[:], 1e-20)
            onesB = xpool.tile([128, B], f32, tag="onesB", name="onesB")
            nc.vector.memset(onesB[:], 1.0)
            capI = xpool.tile([128, B], f32, tag="capI", name="capI")
            nc.vector.memset(capI[:], float(I))

            def squash(s, out_ap, w, tail=False):
                """out = s*|s|/(1+s^2) on [128,w] f32 (|s| via Abs(s+1e-20)).
                tail=True sends the last two muls to Pool (fine when the
                chain overlaps other work); otherwise stay on DVE to avoid
                cross-engine sem hops on the serial poly path."""
                a = spool.tile([128, w], f32, tag="sqa", name="sqa")
                nc.scalar.activation(a[:], s, Act.Abs, bias=epsb[:])
                sq = spool.tile([128, w], f32, tag="sqsq", name="sqsq")
                nc.vector.tensor_mul(sq[:], a[:], a[:])
                u = spool.tile([128, w], f32, tag="squ", name="squ")
                nc.vector.tensor_scalar_add(u[:], sq[:], 1.0)
                r = spool.tile([128, w], f32, tag="sqr", name="sqr")
                nc.vector.reciprocal(r[:], u[:])
                n = spool.tile([128, w], f32, tag="sqn", name="sqn")
                eng = nc.gpsimd if tail else nc.vector
                eng.tensor_tensor(n[:], s, a[:], Alu.mult)
                eng.tensor_tensor(out_ap, n[:], r[:], Alu.mult)

            import contextlib

            def rep_scope():
                if reps == 1:
                    return contextlib.nullcontext(0)
                return tc.For_i(0, reps, 1)

            with rep_scope():
                copy_idx = [0]

                def evac(dst, src, pair):
                    if pair == 0:
                        on_act = copy_idx[0] % act_copy_mod == 0
                    else:
                        on_act = (copy_idx[0] % 12) < p1_act_evac
                    if on_act:
                        nc.scalar.copy(dst, src)
                    else:
                        nc.vector.tensor_copy(dst, src)
                    copy_idx[0] += 1

                nc.scalar.dma_start(densT[:], dens_d[:])
                mean = {}
                for p in range(NPAIR):
                    mean[p] = psmean.tile([128, B], f32, tag=f"mean{p}",
                                          name=f"mean{p}")

                def gen_pair(p):
                    for c in range(NCHUNK):
                        if p == 0:
                            # odd chunks via the ACT HWDGE queue: SP and ACT
                            # queues transfer in parallel on HW, shortening
                            # the pair-0 critical DMA stream
                            q_in = (nc.scalar.dma_start
                                    if c % 2 == 1 and c // 2 < inpd_act
                                    else nc.sync.dma_start)
                            q_in(inpD[:, c * CHUNK:(c + 1) * CHUNK, :],
                                 inpblk_d[c])
                        wsup = wpool.tile([128, CHUNK, 128], bf16,
                                          tag="wsup", name="wsup")
                        q_w = (nc.gpsimd.dma_start
                               if p == 1 and c % 2 == 1 and c // 2 < wsup1_pool
                               else nc.sync.dma_start)
                        q_w(wsup[:], wslab_d[p, c])
                        for g2 in range(CHUNK // GRP):
                            psx = psum.tile([128, GRP, 128], f32,
                                            tag="psx", name="psx")
                            for j in range(GRP):
                                cb = g2 * GRP + j
                                blk = c * CHUNK + cb
                                nc.tensor.matmul(
                                    psx[:, j, :], wsup[:, cb, :],
                                    inpD[:, blk, :], start=True, stop=True)
                                nc.tensor.matmul(
                                    mean[p][:], wsup[:, cb, :],
                                    densT[:, blk, :],
                                    start=(blk == 0),
                                    stop=(blk == NBLK - 1))
                            blk0 = c * CHUNK + g2 * GRP
                            evac(X[p][:, blk0:blk0 + GRP, :], psx[:], p)

                def s2_ops(p, S2, b, h):
                    """S2 half-slice h in {0,1}: S2[:, b] (+)= Σ_half x_hat^2.
                    Halves let the first half start when only X[p][:72] is
                    evacuated; caller combines the two accum tiles."""
                    lo, hi = h * (NBLK // 2), (h + 1) * (NBLK // 2)
                    xv = X[p][:, lo:hi, b * IG:(b + 1) * IG]
                    n_act = q0 if p == 0 else q1
                    if b < n_act:
                        E = epool.tile([128, NBLK // 2, IG], bf16,
                                       tag="S2e", name="S2e")
                        nc.scalar.activation(E[:], xv, Act.Square,
                                             accum_out=S2[:, b:b + 1])
                    else:
                        P = ppool.tile([128, NBLK // 2, IG], bf16, tag="Ph",
                                       name="Ph")
                        nc.vector.scalar_tensor_tensor(
                            P[:], xv, 1.0, xv, Alu.mult, Alu.mult,
                            accum_out=S2[:, b:b + 1])

                def poly(p, S2a, S2b):
                    """V1 for pair p from moments S1 (psum mean) and S2."""
                    S2 = spool.tile([128, B], f32, tag=f"S2f{p}",
                                    name=f"S2f{p}")
                    nc.vector.tensor_add(S2[:], S2a[:], S2b[:])
                    S1 = spool.tile([128, B], f32, tag=f"S1{p}", name=f"S1{p}")
                    nc.vector.tensor_copy(S1[:], mean[p][:])
                    s0 = spool.tile([128, B], f32, tag="s0", name="s0")
                    nc.vector.tensor_scalar_mul(s0[:], S1[:], 1.0 / I)
                    V0 = spool.tile([128, B], f32, tag=f"V0{p}", name=f"V0{p}")
                    squash(s0[:], V0[:], B)
                    # den = I + V0*S1 + 0.5*V0^2*S2 ; num = S1 + V0*S2
                    vs1 = spool.tile([128, B], f32, tag="vs1", name="vs1")
                    nc.vector.tensor_mul(vs1[:], V0[:], S1[:])
                    vh2 = spool.tile([128, B], f32, tag="vh2", name="vh2")
                    nc.vector.scalar_tensor_tensor(
                        vh2[:], V0[:], 0.5, V0[:], Alu.mult, Alu.mult)
                    vs2 = spool.tile([128, B], f32, tag="vs2", name="vs2")
                    nc.vector.tensor_mul(vs2[:], vh2[:], S2[:])
                    d1 = spool.tile([128, B], f32, tag="d1", name="d1")
                    nc.vector.tensor_scalar_add(d1[:], vs1[:], float(I))
                    den = spool.tile([128, B], f32, tag="den", name="den")
                    nc.vector.tensor_add(den[:], d1[:], vs2[:])
                    u = spool.tile([128, B], f32, tag="u", name="u")
                    nc.vector.tensor_mul(u[:], V0[:], S2[:])
                    num = spool.tile([128, B], f32, tag="num", name="num")
                    nc.vector.tensor_add(num[:], S1[:], u[:])
                    rd = spool.tile([128, B], f32, tag="prd", name="prd")
                    nc.vector.reciprocal(rd[:], den[:])
                    s1v = spool.tile([128, B], f32, tag="s1v", name="s1v")
                    nc.vector.tensor_mul(s1v[:], num[:], rd[:])
                    vh = spool.tile([128, B], f32, tag="vh", name="vh")
                    squash(s1v[:], vh[:], B)
                    V1 = spool.tile([128, B], f32, tag=f"V1{p}", name=f"V1{p}")
                    nc.vector.tensor_add(V1[:], V0[:], vh[:])
                    return V1

                def t2_slice(p, b, Vin, denom, numer):
                    xv = X[p][:, :, b * IG:(b + 1) * IG]
                    E = epool.tile([128, NBLK, IG], bf16, tag="E", name="E")
                    nc.scalar.activation(
                        E[:], xv, Act.Exp,
                        scale=Vin[:, b:b + 1],
                        accum_out=denom[:, b:b + 1])
                    P = ppool.tile([128, NBLK, IG], bf16, tag="P", name="P")
                    nc.vector.scalar_tensor_tensor(
                        P[:], E[:], 1.0, xv, Alu.mult, Alu.mult,
                        accum_out=numer[:, b:b + 1])

                def t2_finish(p, denom, numer, h0, h1):
                    w = h1 - h0
                    rd = spool.tile([128, w], f32, tag="rdh", name="rdh")
                    nc.vector.reciprocal(rd[:], denom[:, h0:h1])
                    st = spool.tile([128, w], f32, tag="sth", name="sth")
                    nc.gpsimd.tensor_tensor(st[:], numer[:, h0:h1], rd[:],
                                            Alu.mult)
                    out = spool.tile([128, w], f32, tag="vouth", name="vouth")
                    squash(st[:], out[:], w, tail=True)
                    (nc.gpsimd.dma_start if out_q == 2 else
                     nc.scalar.dma_start)(out_d[p][:, h0:h1], out[:])

                # ---------------- emission ----------------
                gen_pair(0)
                gen_pair(1)

                S2h = [[spool.tile([128, B], f32, tag=f"S2_{p}{h}",
                                   name=f"S2_{p}{h}") for h in range(2)]
                       for p in range(NPAIR)]

                # pair-0 moments (half 0 can start at X0[:72]); pair-1's
                # evacuations land on ACT meanwhile
                for h in range(2):
                    for b in range(B):
                        s2_ops(0, S2h[0][h], b, h)
                V1p0 = poly(0, S2h[0][0], S2h[0][1])

                # iter2 pair 0, interleaved with pair-1 S2 half-slices
                den0 = spool.tile([128, B], f32, tag="den0", name="den0")
                num0 = spool.tile([128, B], f32, tag="num0", name="num0")
                s2q = [(h, b) for h in range(2) for b in range(B)]
                for b in range(B):
                    t2_slice(0, b, V1p0, den0, num0)
                    for _ in range(interleave):
                        if s2q:
                            h1, b1 = s2q.pop(0)
                            s2_ops(1, S2h[1][h1], b1, h1)
                while s2q:
                    h1, b1 = s2q.pop(0)
                    s2_ops(1, S2h[1][h1], b1, h1)
                V1p1 = poly(1, S2h[1][0], S2h[1][1])
                t2_finish(0, den0, num0, 0, B)

                den1 = spool.tile([128, B], f32, tag="den1", name="den1")
                num1 = spool.tile([128, B], f32, tag="num1", name="num1")
                if split_tail:
                    for b in range(B):
                        t2_slice(1, b, V1p1, den1, num1)
                        if b == B // 2 + 1:
                            t2_finish(1, den1, num1, 0, B // 2)
                    t2_finish(1, den1, num1, B // 2, B)
                else:
                    for b in range(B):
                        t2_slice(1, b, V1p1, den1, num1)
                    t2_finish(1, den1, num1, 0, B)

    nc.finalize()
    return nc


def _prep_host(inputs, W):
    """Per-core W slabs, shared block-diagonal input, dense input operand."""
    import ml_dtypes
    bf16 = ml_dtypes.bfloat16

    # wslab[core]: [NPAIR, NCHUNK, (ig,k)=128, cb=CHUNK, (n2,d)=128]
    wslabs = []
    W0 = W[0]  # [N, I, D, DIN]
    for core in range(NCORES):
        Wc = W0[core * NL:(core + 1) * NL]            # [4, I, D, DIN]
        a = Wc.reshape(NPAIR, 2, NCHUNK, CHUNK, IG, D, DIN)
        # axes: pair, n2, chunk, cb, ig, d, k -> pair, chunk, ig, k, cb, n2, d
        bmat = np.ascontiguousarray(a.transpose(0, 2, 4, 6, 3, 1, 5))
        wslabs.append(bmat.reshape(NPAIR, NCHUNK, 128, CHUNK, 128)
                      .astype(bf16))

    # inpblk: [NCHUNK, (ig,k)=128, cb=CHUNK, (b,ig')=128], block-diag in ig
    r = inputs.reshape(B, NCHUNK, CHUNK, IG, DIN).transpose(1, 2, 3, 0, 4)
    # r: [chunk, cb, ig', b, k]
    z = np.zeros((NCHUNK, IG, DIN, CHUNK, B, IG), dtype=np.float32)
    for g in range(IG):
        z[:, g, :, :, :, g] = r[:, :, g, :, :].transpose(0, 3, 1, 2)
    inpblk = z.reshape(NCHUNK, 128, CHUNK, 128).astype(bf16)

    # dens: [(ig,k)=128, blk=144, b=16] dense input for the S1 matmul
    rr = inputs.reshape(B, NCHUNK, CHUNK, IG, DIN)
    dens = np.ascontiguousarray(rr.transpose(3, 4, 1, 2, 0)).reshape(
        128, NBLK, B).astype(bf16)
    return wslabs, inpblk, dens


def kernel(inputs, W):
    from concourse.bass_utils import run_bass_kernel_spmd

    inputs = np.asarray(inputs, dtype=np.float32)
    W = np.asarray(W, dtype=np.float32)

    if "nc" not in _compiled:
        _compiled["nc"] = _build_program()
    nc = _compiled["nc"]

    wslabs, inpblk, dens = _prep_host(inputs, W)
    in_maps = [{"wslab": wslabs[c], "inpblk": inpblk, "dens": dens}
               for c in range(NCORES)]
    res = run_bass_kernel_spmd(nc, in_maps, list(range(NCORES))).results

    out = np.empty((B, N, D), dtype=np.float32)
    for c in range(NCORES):
        o = res[c]["out"]                       # [NPAIR, 128, B]
        o = o.reshape(NPAIR, 2, D, B).transpose(3, 0, 1, 2)  # [B,pair,n2,D]
        out[:, c * NL:(c + 1) * NL, :] = o.reshape(B, NL, D)
    return out[..., None]


# revision 22
# speedup vs baseline: 1.0168x; 1.0168x over previous
"""CapsuleLayer dynamic-routing kernel for Trainium2 (8 NeuronCores).

Problem (hardcoded):
  inputs: [B=16, I=1152, Din=16] f32
  W:      [1, N=32, I=1152, D=64, Din=16] f32
  x_hat = einsum('nidk,bik->bnid', W[0], inputs)        # [B,N,I,D]
  3 routing iterations of per-(b,n,d) softmax over I (size-1-dim squash
  quirk makes everything elementwise in d), output [B,N,D,1] f32.

Key algebra:
  * iter0: softmax(0) uniform -> s0 = mean_i(x_hat); V accumulates squash
    outputs so logits are x_hat * V (never materialized).
  * iter1 via MOMENTS: z = V0*x_hat is small (|z| <= ~2.8 on this data;
    99.9% < 0.77) because V0 = squash(mean_i x_hat / I) ~ O(0.01).  Taylor:
      denom = sum_i e^z   ~= I + V0*S1 + (V0^2/2)*S2
      numer = sum_i x*e^z ~= S1 + V0*S2
    with S1 = sum_i x_hat (the existing mean matmul) and S2 = sum_i x_hat^2
    (one elementwise pass, replacing iter1's STT pass 1:1).  Verified on the
    reference data: final rel err 4.4e-3 vs 4.1e-3 for exact iter1.
  * iter2 exact: E = exp(V1*x_hat) on ACT (accum_out -> denom), DVE
    scalar_tensor_tensor P=(E*1)*x_hat with accum_out -> numer.
  * squash(s) = s*|s|/(1+s^2), with |s| ~= sqrt(s^2+1e-9) via |s+1e-20|.

Mapping (per core; N sharded 8 ways, 4 capsules = 2 "pairs" of (2n x 64d)):
  * x_hat gen: stationary = W slab [(ig,k)=128, (n2,d)=128]; moving =
    block-diagonal input [(ig,k)=128, (b,ig')=128], one matmul per i-block.
    The i-sum S1 accumulates in PSUM from a second matmul per block against
    a DENSE input operand [(ig,k)=128, b=16].
  * PSUM evacuated in [128, 12x128] chunks f32->bf16.  ALL pair-0 copies go
    on ACT (they hide under pair-0's input DMA, and DVE stays free to start
    the S2 pass the moment X half-tiles land); pair-1's copies go 9 ACT / 3
    DVE while DVE runs pair-0's S2/STT stream.
  * S2 slices split DVE (STT x^2 + accum) / ACT (Square activation + accum)
    to balance; pair-1's S2 interleaves with pair-0's iter2 stream.
  * small [128,16] chain ops (poly eval, squash pieces) go to Pool where
    legal (tensor_tensor only); reciprocal stays on DVE; Abs on ACT.
  * outputs DMA via Pool SWDGE; final segment splits the reduce chain in
    batch halves so the first half's squash+DMA hides under the second.
"""

import numpy as np

# ---------------- problem constants (hardcoded per contract) ----------------
B, I, DIN = 16, 1152, 16
N, D = 32, 64
NCORES = 8
NL = N // NCORES        # 4 capsules per core
NPAIR = NL // 2         # 2 capsule-pairs per core (2 n's x 64 d = 128 parts)
IG = 8                  # i's folded into the contraction dim
NBLK = I // IG          # 144 i-blocks
CHUNK = 24              # i-blocks per DMA super-tile
NCHUNK = NBLK // CHUNK  # 6
GRP = 12                # i-blocks per PSUM evacuation tile (3 banks)

_compiled = {}


def _build_program(stage="full", reps=1, act_copy_mod=1, p1_act_evac=9,
                   q1=8, q0=0, epool_bufs=8, ppool_bufs=4, spool_bufs=12,
                   wsup_bufs=4, out_q=2, split_tail=True, use_moments=True,
                   interleave=2, inpd_act=1, wsup1_pool=0):
    import concourse.bacc as bacc
    import concourse.mybir as mybir
    import concourse.tile as tile

    f32 = mybir.dt.float32
    bf16 = mybir.dt.bfloat16
    Alu = mybir.AluOpType
    Act = mybir.ActivationFunctionType

    nc = bacc.Bacc("TRN2", target_bir_lowering=False, debug=False)

    wslab_d = nc.declare_dram_parameter(
        "wslab", [NPAIR, NCHUNK, 128, CHUNK, 128], bf16, isOutput=False)
    inpblk_d = nc.declare_dram_parameter(
        "inpblk", [NCHUNK, 128, CHUNK, 128], bf16, isOutput=False)
    dens_d = nc.declare_dram_parameter(
        "dens", [128, NBLK, B], bf16, isOutput=False)
    out_d = nc.declare_dram_parameter(
        "out", [NPAIR, 128, B], f32, isOutput=True)

    with tile.TileContext(nc) as tc:
        with (
            tc.tile_pool(name="persist", bufs=1) as xpool,
            tc.tile_pool(name="wsup", bufs=wsup_bufs) as wpool,
            tc.tile_pool(name="escr", bufs=epool_bufs) as epool,
            tc.tile_pool(name="pscr", bufs=ppool_bufs) as ppool,
            tc.tile_pool(name="small", bufs=spool_bufs) as spool,
            tc.tile_pool(name="psum", bufs=2, space="PSUM") as psum,
            tc.tile_pool(name="psmean", bufs=1, space="PSUM") as psmean,
        ):
            # X free layout: (blk, col) with col = b*IG + ig
            X = [xpool.tile([128, NBLK, 128], bf16, tag=f"X{p}",
                            name=f"X{p}") for p in range(NPAIR)]
            inpD = xpool.tile([128, NBLK, 128], bf16, tag="inpD", name="inpD")
            densT = xpool.tile([128, NBLK, B], bf16, tag="densT",
                               name="densT")

            epsb = xpool.tile([128, 1], f32, tag="epsb", name="epsb")
            nc.vector.memset(epsb[:], 1e-20)
            onesB = xpool.tile([128, B], f32, tag="onesB", name="onesB")
            nc.vector.memset(onesB[:], 1.0)
            capI = xpool.tile([128, B], f32, tag="capI", name="capI")
            nc.vector.memset(capI[:], float(I))

            def squash(s, out_ap, w, tail=False):
                """out = s*|s|/(1+s^2) on [128,w] f32 (|s| via Abs(s+1e-20)).
                tail=True sends the last two muls to Pool (fine when the
                chain overlaps other work); otherwise stay on DVE to avoid
                cross-engine sem hops on the serial poly path."""
                a = spool.tile([128, w], f32, tag="sqa", name="sqa")
                nc.scalar.activation(a[:], s, Act.Abs, bias=epsb[:])
                sq = spool.tile([128, w], f32, tag="sqsq", name="sqsq")
                nc.vector.tensor_mul(sq[:], a[:], a[:])
                u = spool.tile([128, w], f32, tag="squ", name="squ")
                nc.vector.tensor_scalar_add(u[:], sq[:], 1.0)
                r = spool.tile([128, w], f32, tag="sqr", name="sqr")
                nc.vector.reciprocal(r[:], u[:])
                n = spool.tile([128, w], f32, tag="sqn", name="sqn")
                eng = nc.gpsimd if tail else nc.vector
                eng.tensor_tensor(n[:], s, a[:], Alu.mult)
                eng.tensor_tensor(out_ap, n[:], r[:], Alu.mult)

            import contextlib

            def rep_scope():
                if reps == 1:
                    return contextlib.nullcontext(0)
                return tc.For_i(0, reps, 1)

            with rep_scope():
                copy_idx = [0]

                def evac(dst, src, pair):
                    if pair == 0:
                        on_act = copy_idx[0] % act_copy_mod == 0
                    else:
                        on_act = (copy_idx[0] % 12) < p1_act_evac
                    if on_act:
                        nc.scalar.copy(dst, src)
                    else:
                        nc.vector.tensor_copy(dst, src)
                    copy_idx[0] += 1

                nc.scalar.dma_start(densT[:], dens_d[:])
                mean = {}
                for p in range(NPAIR):
                    mean[p] = psmean.tile([128, B], f32, tag=f"mean{p}",
                                          name=f"mean{p}")

                def gen_pair(p):
                    for c in range(NCHUNK):
                        if p == 0:
                            # odd chunks via the ACT HWDGE queue: SP and ACT
                            # queues transfer in parallel on HW, shortening
                            # the pair-0 critical DMA stream
                            q_in = (nc.scalar.dma_start
                                    if c % 2 == 1 and c // 2 < inpd_act
                                    else nc.sync.dma_start)
                            q_in(inpD[:, c * CHUNK:(c + 1) * CHUNK, :],
                                 inpblk_d[c])
                        wsup = wpool.tile([128, CHUNK, 128], bf16,
                                          tag="wsup", name="wsup")
                        q_w = (nc.gpsimd.dma_start
                               if p == 1 and c % 2 == 1 and c // 2 < wsup1_pool
                               else nc.sync.dma_start)
                        q_w(wsup[:], wslab_d[p, c])
                        for g2 in range(CHUNK // GRP):
                            psx = psum.tile([128, GRP, 128], f32,
                                            tag="psx", name="psx")
                            for j in range(GRP):
                                cb = g2 * GRP + j
                                blk = c * CHUNK + cb
                                nc.tensor.matmul(
                                    psx[:, j, :], wsup[:, cb, :],
                                    inpD[:, blk, :], start=True, stop=True)
                                nc.tensor.matmul(
                                    mean[p][:], wsup[:, cb, :],
                                    densT[:, blk, :],
                                    start=(blk == 0),
                                    stop=(blk == NBLK - 1))
                            blk0 = c * CHUNK + g2 * GRP
                            evac(X[p][:, blk0:blk0 + GRP, :], psx[:], p)

                def s2_ops(p, S2, b, h):
                    """S2 half-slice h in {0,1}: S2[:, b] (+)= Σ_half x_hat^2.
                    Halves let the first half start when only X[p][:72] is
                    evacuated; caller combines the two accum tiles."""
                    lo, hi = h * (NBLK // 2), (h + 1) * (NBLK // 2)
                    xv = X[p][:, lo:hi, b * IG:(b + 1) * IG]
                    n_act = q0 if p == 0 else q1
                    if b < n_act:
                        E = epool.tile([128, NBLK // 2, IG], bf16,
                                       tag="S2e", name="S2e")
                        nc.scalar.activation(E[:], xv, Act.Square,
                                             accum_out=S2[:, b:b + 1])
                    else:
                        P = ppool.tile([128, NBLK // 2, IG], bf16, tag="Ph",
                                       name="Ph")
                        nc.vector.scalar_tensor_tensor(
                            P[:], xv, 1.0, xv, Alu.mult, Alu.mult,
                            accum_out=S2[:, b:b + 1])

                def poly(p, S2a, S2b):
                    """V1 for pair p from moments S1 (psum mean) and S2."""
                    S2 = spool.tile([128, B], f32, tag=f"S2f{p}",
                                    name=f"S2f{p}")
                    nc.vector.tensor_add(S2[:], S2a[:], S2b[:])
                    S1 = spool.tile([128, B], f32, tag=f"S1{p}", name=f"S1{p}")
                    nc.vector.tensor_copy(S1[:], mean[p][:])
                    s0 = spool.tile([128, B], f32, tag="s0", name="s0")
                    nc.vector.tensor_scalar_mul(s0[:], S1[:], 1.0 / I)
                    V0 = spool.tile([128, B], f32, tag=f"V0{p}", name=f"V0{p}")
                    squash(s0[:], V0[:], B)
                    # den = I + V0*S1 + 0.5*V0^2*S2 ; num = S1 + V0*S2
                    vs1 = spool.tile([128, B], f32, tag="vs1", name="vs1")
                    nc.vector.tensor_mul(vs1[:], V0[:], S1[:])
                    vh2 = spool.tile([128, B], f32, tag="vh2", name="vh2")
                    nc.vector.scalar_tensor_tensor(
                        vh2[:], V0[:], 0.5, V0[:], Alu.mult, Alu.mult)
                    vs2 = spool.tile([128, B], f32, tag="vs2", name="vs2")
                    nc.vector.tensor_mul(vs2[:], vh2[:], S2[:])
                    d1 = spool.tile([128, B], f32, tag="d1", name="d1")
                    nc.vector.tensor_scalar_add(d1[:], vs1[:], float(I))
                    den = spool.tile([128, B], f32, tag="den", name="den")
                    nc.vector.tensor_add(den[:], d1[:], vs2[:])
                    u = spool.tile([128, B], f32, tag="u", name="u")
                    nc.vector.tensor_mul(u[:], V0[:], S2[:])
                    num = spool.tile([128, B], f32, tag="num", name="num")
                    nc.vector.tensor_add(num[:], S1[:], u[:])
                    rd = spool.tile([128, B], f32, tag="prd", name="prd")
                    nc.vector.reciprocal(rd[:], den[:])
                    s1v = spool.tile([128, B], f32, tag="s1v", name="s1v")
                    nc.vector.tensor_mul(s1v[:], num[:], rd[:])
                    vh = spool.tile([128, B], f32, tag="vh", name="vh")
                    squash(s1v[:], vh[:], B)
                    V1 = spool.tile([128, B], f32, tag=f"V1{p}", name=f"V1{p}")
                    nc.vector.tensor_add(V1[:], V0[:], vh[:])
                    return V1

                def t2_slice(p, b, Vin, denom, numer):
                    xv = X[p][:, :, b * IG:(b + 1) * IG]
                    E = epool.tile([128, NBLK, IG], bf16, tag="E", name="E")
                    nc.scalar.activation(
                        E[:], xv, Act.Exp,
                        scale=Vin[:, b:b + 1],
                        accum_out=denom[:, b:b + 1])
                    P = ppool.tile([128, NBLK, IG], bf16, tag="P", name="P")
                    nc.vector.scalar_tensor_tensor(
                        P[:], E[:], 1.0, xv, Alu.mult, Alu.mult,
                        accum_out=numer[:, b:b + 1])

                def t2_finish(p, denom, numer, h0, h1):
                    w = h1 - h0
                    rd = spool.tile([128, w], f32, tag="rdh", name="rdh")
                    nc.vector.reciprocal(rd[:], denom[:, h0:h1])
                    st = spool.tile([128, w], f32, tag="sth", name="sth")
                    nc.gpsimd.tensor_tensor(st[:], numer[:, h0:h1], rd[:],
                                            Alu.mult)
                    out = spool.tile([128, w], f32, tag="vouth", name="vouth")
                    squash(st[:], out[:], w, tail=True)
                    (nc.gpsimd.dma_start if out_q == 2 else
                     nc.scalar.dma_start)(out_d[p][:, h0:h1], out[:])

                # ---------------- emission ----------------
                gen_pair(0)
                gen_pair(1)

                S2h = [[spool.tile([128, B], f32, tag=f"S2_{p}{h}",
                                   name=f"S2_{p}{h}") for h in range(2)]
                       for p in range(NPAIR)]

                # pair-0 moments (half 0 can start at X0[:72]); pair-1's
                # evacuations land on ACT meanwhile
                for h in range(2):
                    for b in range(B):
                        s2_ops(0, S2h[0][h], b, h)
                V1p0 = poly(0, S2h[0][0], S2h[0][1])

                # iter2 pair 0, interleaved with pair-1 S2 half-slices
                den0 = spool.tile([128, B], f32, tag="den0", name="den0")
                num0 = spool.tile([128, B], f32, tag="num0", name="num0")
                s2q = [(h, b) for h in range(2) for b in range(B)]
                for b in range(B):
                    t2_slice(0, b, V1p0, den0, num0)
                    for _ in range(interleave):
                        if s2q:
                            h1, b1 = s2q.pop(0)
                            s2_ops(1, S2h[1][h1], b1, h1)
                while s2q:
                    h1, b1 = s2q.pop(0)
                    s2_ops(1, S2h[1][h1], b1, h1)
                V1p1 = poly(1, S2h[1][0], S2h[1][1])
                t2_finish(0, den0, num0, 0, B)

                den1 = spool.tile([128, B], f32, tag="den1", name="den1")
                num1 = spool.tile([128, B], f32, tag="num1", name="num1")
                if split_tail:
                    for b in range(B):
                        t2_slice(1, b, V1p1, den1, num1)
                        if b == B // 2 + 1:
                            t2_finish(1, den1, num1, 0, B // 2)
                    t2_finish(1, den1, num1, B // 2, B)
                else:
                    for b in range(B):
                        t2_slice(1, b, V1p1, den1, num1)
                    t2_finish(1, den1, num1, 0, B)

    nc.finalize()
    return nc


def _prep_host(inputs, W):
    """Per-core W slabs, shared block-diagonal input, dense input operand."""
    import ml_dtypes
    bf16 = ml_dtypes.bfloat16

    # wslab[core]: [NPAIR, NCHUNK, (ig,k)=128, cb=CHUNK, (n2,d)=128]
    wslabs = []
    W0 = W[0]  # [N, I, D, DIN]
    for core in range(NCORES):
        Wc = W0[core * NL:(core + 1) * NL]            # [4, I, D, DIN]
        a = Wc.reshape(NPAIR, 2, NCHUNK, CHUNK, IG, D, DIN)
        # axes: pair, n2, chunk, cb, ig, d, k -> pair, chunk, ig, k, cb, n2, d
        bmat = np.ascontiguousarray(a.transpose(0, 2, 4, 6, 3, 1, 5))
        wslabs.append(bmat.reshape(NPAIR, NCHUNK, 128, CHUNK, 128)
                      .astype(bf16))

    # inpblk: [NCHUNK, (ig,k)=128, cb=CHUNK, (b,ig')=128], block-diag in ig
    r = inputs.reshape(B, NCHUNK, CHUNK, IG, DIN).transpose(1, 2, 3, 0, 4)
    # r: [chunk, cb, ig', b, k]
    z = np.zeros((NCHUNK, IG, DIN, CHUNK, B, IG), dtype=np.float32)
    for g in range(IG):
        z[:, g, :, :, :, g] = r[:, :, g, :, :].transpose(0, 3, 1, 2)
    inpblk = z.reshape(NCHUNK, 128, CHUNK, 128).astype(bf16)

    # dens: [(ig,k)=128, blk=144, b=16] dense input for the S1 matmul
    rr = inputs.reshape(B, NCHUNK, CHUNK, IG, DIN)
    dens = np.ascontiguousarray(rr.transpose(3, 4, 1, 2, 0)).reshape(
        128, NBLK, B).astype(bf16)
    return wslabs, inpblk, dens


def kernel(inputs, W):
    from concourse.bass_utils import run_bass_kernel_spmd

    inputs = np.asarray(inputs, dtype=np.float32)
    W = np.asarray(W, dtype=np.float32)

    if "nc" not in _compiled:
        _compiled["nc"] = _build_program()
    nc = _compiled["nc"]

    wslabs, inpblk, dens = _prep_host(inputs, W)
    in_maps = [{"wslab": wslabs[c], "inpblk": inpblk, "dens": dens}
               for c in range(NCORES)]
    res = run_bass_kernel_spmd(nc, in_maps, list(range(NCORES))).results

    out = np.empty((B, N, D), dtype=np.float32)
    for c in range(NCORES):
        o = res[c]["out"]                       # [NPAIR, 128, B]
        o = o.reshape(NPAIR, 2, D, B).transpose(3, 0, 1, 2)  # [B,pair,n2,D]
        out[:, c * NL:(c + 1) * NL, :] = o.reshape(B, NL, D)
    return out[..., None]


# revision 24
# speedup vs baseline: 1.0360x; 1.0188x over previous
"""CapsuleLayer dynamic-routing kernel for Trainium2 (8 NeuronCores).

Problem (hardcoded):
  inputs: [B=16, I=1152, Din=16] f32
  W:      [1, N=32, I=1152, D=64, Din=16] f32
  x_hat = einsum('nidk,bik->bnid', W[0], inputs)        # [B,N,I,D]
  3 routing iterations of per-(b,n,d) softmax over I (size-1-dim squash
  quirk makes everything elementwise in d), output [B,N,D,1] f32.

Key algebra:
  * iter0: softmax(0) uniform -> s0 = mean_i(x_hat); V accumulates squash
    outputs so logits are x_hat * V (never materialized).
  * iter1 via MOMENTS: z = V0*x_hat is small (|z| <= ~2.8 on this data;
    99.9% < 0.77) because V0 = squash(mean_i x_hat / I) ~ O(0.01).  Taylor:
      denom = sum_i e^z   ~= I + V0*S1 + (V0^2/2)*S2
      numer = sum_i x*e^z ~= S1 + V0*S2
    with S1 = sum_i x_hat (the existing mean matmul) and S2 = sum_i x_hat^2
    (one elementwise pass, replacing iter1's STT pass 1:1).  Verified on the
    reference data: final rel err 4.4e-3 vs 4.1e-3 for exact iter1.
  * iter2 exact: E = exp(V1*x_hat) on ACT (accum_out -> denom), DVE
    scalar_tensor_tensor P=(E*1)*x_hat with accum_out -> numer.
  * squash(s) = s*|s|/(1+s^2), with |s| ~= sqrt(s^2+1e-9) via |s+1e-20|.

Mapping (per core; N sharded 8 ways, 4 capsules = 2 "pairs" of (2n x 64d)):
  * x_hat gen: stationary = W slab [(ig,k)=128, (n2,d)=128]; moving =
    block-diagonal input [(ig,k)=128, (b,ig')=128], one matmul per i-block.
    The i-sum S1 accumulates in PSUM from a second matmul per block against
    a DENSE input operand [(ig,k)=128, b=16].
  * PSUM evacuated in [128, 12x128] chunks f32->bf16.  ALL pair-0 copies go
    on ACT (they hide under pair-0's input DMA, and DVE stays free to start
    the S2 pass the moment X half-tiles land); pair-1's copies go 9 ACT / 3
    DVE while DVE runs pair-0's S2/STT stream.
  * S2 slices split DVE (STT x^2 + accum) / ACT (Square activation + accum)
    to balance; pair-1's S2 interleaves with pair-0's iter2 stream.
  * small [128,16] chain ops (poly eval, squash pieces) go to Pool where
    legal (tensor_tensor only); reciprocal stays on DVE; Abs on ACT.
  * outputs DMA via Pool SWDGE; final segment splits the reduce chain in
    batch halves so the first half's squash+DMA hides under the second.
"""

import numpy as np

# ---------------- problem constants (hardcoded per contract) ----------------
B, I, DIN = 16, 1152, 16
N, D = 32, 64
NCORES = 8
NL = N // NCORES        # 4 capsules per core
NPAIR = NL // 2         # 2 capsule-pairs per core (2 n's x 64 d = 128 parts)
IG = 8                  # i's folded into the contraction dim
NBLK = I // IG          # 144 i-blocks
CHUNK = 24              # i-blocks per DMA super-tile
NCHUNK = NBLK // CHUNK  # 6
GRP = 12                # i-blocks per PSUM evacuation tile (3 banks)

_compiled = {}


def _build_program(stage="full", reps=1, act_copy_mod=1, p1_act_evac=9,
                   q1=8, q0=0, epool_bufs=8, ppool_bufs=4, spool_bufs=12,
                   wsup_bufs=4, out_q=2, split_tail=True, use_moments=True,
                   interleave=3, inpd_act=1, wsup1_pool=0,
                   p0_dve_tail=0, s2q_act_first=True):
    import concourse.bacc as bacc
    import concourse.mybir as mybir
    import concourse.tile as tile

    f32 = mybir.dt.float32
    bf16 = mybir.dt.bfloat16
    Alu = mybir.AluOpType
    Act = mybir.ActivationFunctionType

    nc = bacc.Bacc("TRN2", target_bir_lowering=False, debug=False)

    wslab_d = nc.declare_dram_parameter(
        "wslab", [NPAIR, NCHUNK, 128, CHUNK, 128], bf16, isOutput=False)
    inpblk_d = nc.declare_dram_parameter(
        "inpblk", [NCHUNK, 128, CHUNK, 128], bf16, isOutput=False)
    dens_d = nc.declare_dram_parameter(
        "dens", [128, NBLK, B], bf16, isOutput=False)
    out_d = nc.declare_dram_parameter(
        "out", [NPAIR, 128, B], f32, isOutput=True)

    with tile.TileContext(nc) as tc:
        with (
            tc.tile_pool(name="persist", bufs=1) as xpool,
            tc.tile_pool(name="wsup", bufs=wsup_bufs) as wpool,
            tc.tile_pool(name="escr", bufs=epool_bufs) as epool,
            tc.tile_pool(name="pscr", bufs=ppool_bufs) as ppool,
            tc.tile_pool(name="small", bufs=spool_bufs) as spool,
            tc.tile_pool(name="psum", bufs=2, space="PSUM") as psum,
            tc.tile_pool(name="psmean", bufs=1, space="PSUM") as psmean,
        ):
            # X free layout: (blk, col) with col = b*IG + ig
            X = [xpool.tile([128, NBLK, 128], bf16, tag=f"X{p}",
                            name=f"X{p}") for p in range(NPAIR)]
            inpD = xpool.tile([128, NBLK, 128], bf16, tag="inpD", name="inpD")
            densT = xpool.tile([128, NBLK, B], bf16, tag="densT",
                               name="densT")

            epsb = xpool.tile([128, 1], f32, tag="epsb", name="epsb")
            nc.vector.memset(epsb[:], 1e-20)
            onesB = xpool.tile([128, B], f32, tag="onesB", name="onesB")
            nc.vector.memset(onesB[:], 1.0)
            capI = xpool.tile([128, B], f32, tag="capI", name="capI")
            nc.vector.memset(capI[:], float(I))

            def squash(s, out_ap, w, tail=False):
                """out = s*|s|/(1+s^2) on [128,w] f32 (|s| via Abs(s+1e-20)).
                tail=True sends the last two muls to Pool (fine when the
                chain overlaps other work); otherwise stay on DVE to avoid
                cross-engine sem hops on the serial poly path."""
                a = spool.tile([128, w], f32, tag="sqa", name="sqa")
                nc.scalar.activation(a[:], s, Act.Abs, bias=epsb[:])
                sq = spool.tile([128, w], f32, tag="sqsq", name="sqsq")
                nc.vector.tensor_mul(sq[:], a[:], a[:])
                u = spool.tile([128, w], f32, tag="squ", name="squ")
                nc.vector.tensor_scalar_add(u[:], sq[:], 1.0)
                r = spool.tile([128, w], f32, tag="sqr", name="sqr")
                nc.vector.reciprocal(r[:], u[:])
                n = spool.tile([128, w], f32, tag="sqn", name="sqn")
                eng = nc.gpsimd if tail else nc.vector
                eng.tensor_tensor(n[:], s, a[:], Alu.mult)
                eng.tensor_tensor(out_ap, n[:], r[:], Alu.mult)

            import contextlib

            def rep_scope():
                if reps == 1:
                    return contextlib.nullcontext(0)
                return tc.For_i(0, reps, 1)

            with rep_scope():
                copy_idx = [0]

                def evac(dst, src, pair):
                    if pair == 0:
                        # last p0_dve_tail copies go to the idle DVE so X0
                        # completes sooner (ACT may still be draining its
                        # evac backlog when the final matmuls land)
                        on_act = (copy_idx[0] % act_copy_mod == 0
                                  and copy_idx[0] < 12 - p0_dve_tail)
                    else:
                        on_act = (copy_idx[0] % 12) < p1_act_evac
                    if on_act:
                        nc.scalar.copy(dst, src)
                    else:
                        nc.vector.tensor_copy(dst, src)
                    copy_idx[0] += 1

                nc.scalar.dma_start(densT[:], dens_d[:])
                mean = {}
                for p in range(NPAIR):
                    mean[p] = psmean.tile([128, B], f32, tag=f"mean{p}",
                                          name=f"mean{p}")

                def gen_pair(p):
                    for c in range(NCHUNK):
                        if p == 0:
                            # odd chunks via the ACT HWDGE queue: SP and ACT
                            # queues transfer in parallel on HW, shortening
                            # the pair-0 critical DMA stream
                            q_in = (nc.scalar.dma_start
                                    if c % 2 == 1 and c // 2 < inpd_act
                                    else nc.sync.dma_start)
                            q_in(inpD[:, c * CHUNK:(c + 1) * CHUNK, :],
                                 inpblk_d[c])
                        wsup = wpool.tile([128, CHUNK, 128], bf16,
                                          tag="wsup", name="wsup")
                        q_w = (nc.gpsimd.dma_start
                               if p == 1 and c % 2 == 1 and c // 2 < wsup1_pool
                               else nc.sync.dma_start)
                        q_w(wsup[:], wslab_d[p, c])
                        for g2 in range(CHUNK // GRP):
                            psx = psum.tile([128, GRP, 128], f32,
                                            tag="psx", name="psx")
                            for j in range(GRP):
                                cb = g2 * GRP + j
                                blk = c * CHUNK + cb
                                nc.tensor.matmul(
                                    psx[:, j, :], wsup[:, cb, :],
                                    inpD[:, blk, :], start=True, stop=True)
                                nc.tensor.matmul(
                                    mean[p][:], wsup[:, cb, :],
                                    densT[:, blk, :],
                                    start=(blk == 0),
                                    stop=(blk == NBLK - 1))
                            blk0 = c * CHUNK + g2 * GRP
                            evac(X[p][:, blk0:blk0 + GRP, :], psx[:], p)

                def s2_ops(p, S2, b, h):
                    """S2 half-slice h in {0,1}: S2[:, b] (+)= Σ_half x_hat^2.
                    Halves let the first half start when only X[p][:72] is
                    evacuated; caller combines the two accum tiles."""
                    lo, hi = h * (NBLK // 2), (h + 1) * (NBLK // 2)
                    xv = X[p][:, lo:hi, b * IG:(b + 1) * IG]
                    n_act = q0 if p == 0 else q1
                    if b < n_act:
                        E = epool.tile([128, NBLK // 2, IG], bf16,
                                       tag="S2e", name="S2e")
                        nc.scalar.activation(E[:], xv, Act.Square,
                                             accum_out=S2[:, b:b + 1])
                    else:
                        P = ppool.tile([128, NBLK // 2, IG], bf16, tag="Ph",
                                       name="Ph")
                        nc.vector.scalar_tensor_tensor(
                            P[:], xv, 1.0, xv, Alu.mult, Alu.mult,
                            accum_out=S2[:, b:b + 1])

                def poly(p, S2a, S2b):
                    """V1 for pair p from moments S1 (psum mean) and S2."""
                    S2 = spool.tile([128, B], f32, tag=f"S2f{p}",
                                    name=f"S2f{p}")
                    nc.vector.tensor_add(S2[:], S2a[:], S2b[:])
                    S1 = spool.tile([128, B], f32, tag=f"S1{p}", name=f"S1{p}")
                    nc.vector.tensor_copy(S1[:], mean[p][:])
                    s0 = spool.tile([128, B], f32, tag="s0", name="s0")
                    nc.vector.tensor_scalar_mul(s0[:], S1[:], 1.0 / I)
                    V0 = spool.tile([128, B], f32, tag=f"V0{p}", name=f"V0{p}")
                    squash(s0[:], V0[:], B)
                    # den = I + V0*S1 + 0.5*V0^2*S2 ; num = S1 + V0*S2
                    vs1 = spool.tile([128, B], f32, tag="vs1", name="vs1")
                    nc.vector.tensor_mul(vs1[:], V0[:], S1[:])
                    vh2 = spool.tile([128, B], f32, tag="vh2", name="vh2")
                    nc.vector.scalar_tensor_tensor(
                        vh2[:], V0[:], 0.5, V0[:], Alu.mult, Alu.mult)
                    vs2 = spool.tile([128, B], f32, tag="vs2", name="vs2")
                    nc.vector.tensor_mul(vs2[:], vh2[:], S2[:])
                    d1 = spool.tile([128, B], f32, tag="d1", name="d1")
                    nc.vector.tensor_scalar_add(d1[:], vs1[:], float(I))
                    den = spool.tile([128, B], f32, tag="den", name="den")
                    nc.vector.tensor_add(den[:], d1[:], vs2[:])
                    u = spool.tile([128, B], f32, tag="u", name="u")
                    nc.vector.tensor_mul(u[:], V0[:], S2[:])
                    num = spool.tile([128, B], f32, tag="num", name="num")
                    nc.vector.tensor_add(num[:], S1[:], u[:])
                    rd = spool.tile([128, B], f32, tag="prd", name="prd")
                    nc.vector.reciprocal(rd[:], den[:])
                    s1v = spool.tile([128, B], f32, tag="s1v", name="s1v")
                    nc.vector.tensor_mul(s1v[:], num[:], rd[:])
                    vh = spool.tile([128, B], f32, tag="vh", name="vh")
                    squash(s1v[:], vh[:], B)
                    V1 = spool.tile([128, B], f32, tag=f"V1{p}", name=f"V1{p}")
                    nc.vector.tensor_add(V1[:], V0[:], vh[:])
                    return V1

                def t2_slice(p, b, Vin, denom, numer):
                    xv = X[p][:, :, b * IG:(b + 1) * IG]
                    E = epool.tile([128, NBLK, IG], bf16, tag="E", name="E")
                    nc.scalar.activation(
                        E[:], xv, Act.Exp,
                        scale=Vin[:, b:b + 1],
                        accum_out=denom[:, b:b + 1])
                    P = ppool.tile([128, NBLK, IG], bf16, tag="P", name="P")
                    nc.vector.scalar_tensor_tensor(
                        P[:], E[:], 1.0, xv, Alu.mult, Alu.mult,
                        accum_out=numer[:, b:b + 1])

                def t2_finish(p, denom, numer, h0, h1):
                    w = h1 - h0
                    rd = spool.tile([128, w], f32, tag="rdh", name="rdh")
                    nc.vector.reciprocal(rd[:], denom[:, h0:h1])
                    st = spool.tile([128, w], f32, tag="sth", name="sth")
                    nc.gpsimd.tensor_tensor(st[:], numer[:, h0:h1], rd[:],
                                            Alu.mult)
                    out = spool.tile([128, w], f32, tag="vouth", name="vouth")
                    squash(st[:], out[:], w, tail=True)
                    (nc.gpsimd.dma_start if out_q == 2 else
                     nc.scalar.dma_start)(out_d[p][:, h0:h1], out[:])

                # ---------------- emission ----------------
                gen_pair(0)
                gen_pair(1)

                S2h = [[spool.tile([128, B], f32, tag=f"S2_{p}{h}",
                                   name=f"S2_{p}{h}") for h in range(2)]
                       for p in range(NPAIR)]

                # pair-0 moments (half 0 can start at X0[:72]); pair-1's
                # evacuations land on ACT meanwhile
                for h in range(2):
                    for b in range(B):
                        s2_ops(0, S2h[0][h], b, h)
                V1p0 = poly(0, S2h[0][0], S2h[0][1])

                # iter2 pair 0, interleaved with pair-1 S2 half-slices
                den0 = spool.tile([128, B], f32, tag="den0", name="den0")
                num0 = spool.tile([128, B], f32, tag="num0", name="num0")
                s2q = [(h, b) for h in range(2) for b in range(B)]
                if s2q_act_first:
                    # ACT's Square slices (b < q1) first: ACT absorbs them
                    # early between exps; DVE's share lands at phase-2 end
                    # when DVE has slack, so poly(1) starts sooner
                    s2q.sort(key=lambda hb: hb[1] >= q1)
                for b in range(B):
                    t2_slice(0, b, V1p0, den0, num0)
                    for _ in range(interleave):
                        if s2q:
                            h1, b1 = s2q.pop(0)
                            s2_ops(1, S2h[1][h1], b1, h1)
                while s2q:
                    h1, b1 = s2q.pop(0)
                    s2_ops(1, S2h[1][h1], b1, h1)
                V1p1 = poly(1, S2h[1][0], S2h[1][1])
                t2_finish(0, den0, num0, 0, B)

                den1 = spool.tile([128, B], f32, tag="den1", name="den1")
                num1 = spool.tile([128, B], f32, tag="num1", name="num1")
                if split_tail:
                    for b in range(B):
                        t2_slice(1, b, V1p1, den1, num1)
                        if b == B // 2 + 1:
                            t2_finish(1, den1, num1, 0, B // 2)
                    t2_finish(1, den1, num1, B // 2, B)
                else:
                    for b in range(B):
                        t2_slice(1, b, V1p1, den1, num1)
                    t2_finish(1, den1, num1, 0, B)

    nc.finalize()
    return nc


def _prep_host(inputs, W):
    """Per-core W slabs, shared block-diagonal input, dense input operand."""
    import ml_dtypes
    bf16 = ml_dtypes.bfloat16

    # wslab[core]: [NPAIR, NCHUNK, (ig,k)=128, cb=CHUNK, (n2,d)=128]
    wslabs = []
    W0 = W[0]  # [N, I, D, DIN]
    for core in range(NCORES):
        Wc = W0[core * NL:(core + 1) * NL]            # [4, I, D, DIN]
        a = Wc.reshape(NPAIR, 2, NCHUNK, CHUNK, IG, D, DIN)
        # axes: pair, n2, chunk, cb, ig, d, k -> pair, chunk, ig, k, cb, n2, d
        bmat = np.ascontiguousarray(a.transpose(0, 2, 4, 6, 3, 1, 5))
        wslabs.append(bmat.reshape(NPAIR, NCHUNK, 128, CHUNK, 128)
                      .astype(bf16))

    # inpblk: [NCHUNK, (ig,k)=128, cb=CHUNK, (b,ig')=128], block-diag in ig
    r = inputs.reshape(B, NCHUNK, CHUNK, IG, DIN).transpose(1, 2, 3, 0, 4)
    # r: [chunk, cb, ig', b, k]
    z = np.zeros((NCHUNK, IG, DIN, CHUNK, B, IG), dtype=np.float32)
    for g in range(IG):
        z[:, g, :, :, :, g] = r[:, :, g, :, :].transpose(0, 3, 1, 2)
    inpblk = z.reshape(NCHUNK, 128, CHUNK, 128).astype(bf16)

    # dens: [(ig,k)=128, blk=144, b=16] dense input for the S1 matmul
    rr = inputs.reshape(B, NCHUNK, CHUNK, IG, DIN)
    dens = np.ascontiguousarray(rr.transpose(3, 4, 1, 2, 0)).reshape(
        128, NBLK, B).astype(bf16)
    return wslabs, inpblk, dens


def kernel(inputs, W):
    from concourse.bass_utils import run_bass_kernel_spmd

    inputs = np.asarray(inputs, dtype=np.float32)
    W = np.asarray(W, dtype=np.float32)

    if "nc" not in _compiled:
        _compiled["nc"] = _build_program()
    nc = _compiled["nc"]

    wslabs, inpblk, dens = _prep_host(inputs, W)
    in_maps = [{"wslab": wslabs[c], "inpblk": inpblk, "dens": dens}
               for c in range(NCORES)]
    res = run_bass_kernel_spmd(nc, in_maps, list(range(NCORES))).results

    out = np.empty((B, N, D), dtype=np.float32)
    for c in range(NCORES):
        o = res[c]["out"]                       # [NPAIR, 128, B]
        o = o.reshape(NPAIR, 2, D, B).transpose(3, 0, 1, 2)  # [B,pair,n2,D]
        out[:, c * NL:(c + 1) * NL, :] = o.reshape(B, NL, D)
    return out[..., None]
